# revision 22
# baseline (speedup 1.0000x reference)
"""HONU order-3 kernel for 8 TRN2 NeuronCores — raw bass (no TileContext).

Math: out[b] = sum_{i<=j<=k} w_ijk * xf_i * xf_j * xf_k,  xf = [1, x] (127 feats).

Restructuring: group combos by pair (i,j) (lex order => per-pair weights are a
contiguous slice of `weights`).  Let W[(i,j), k] = w_ijk for k>=j (0 otherwise).
Then  Z[b,(i,j)] = sum_k W[(i,j),k] * xf[b,k]   (a dense matmul), and
      out[b]     = sum_i xf_i * sum_{j>=i} xf_j * Z[b,(i,j)]
mapped onto one fused scalar_tensor_tensor per i-row and batch tile:
      g[:, t] = sum_j ((Z * xf_i) * xf_j).

Sharding: pair-rows i are dealt round-robin to the 8 cores (core c gets rows
i = 8t + c, t = 0..15); every core runs the same (SPMD) program and returns a
[128, 2] partial (batch-tile column-major) that the host sums across cores.

Why raw bass: the TileContext version spent ~10us in framework preamble +
semaphore-teardown epilogue and ~14us moving 837KB as sub-1KB DMA packets.
Here: 5 consolidated DMAs (>=512B per-partition rows), 7 manual semaphores,
no nc.Block() final barrier (lets the fixed walrus per-engine sem-file-reset
epilogue start as early as possible), bf16 matmul operands (halves weight
DMA; full-rate PE), fp32 elementwise (STT has no 2x bf16 mode - measured),
tile-0 chunks 0-1 consumed straight from PSUM to cut pipeline-fill latency.
Measured: 28.5us (Tile baseline) -> ~20.5us.
"""

import numpy as np
import ml_dtypes

import concourse.bass as bass
import concourse.bacc as bacc
import concourse.mybir as mybir
from concourse.bass_utils import run_bass_kernel_spmd

F32 = mybir.dt.float32
BF16 = mybir.dt.bfloat16
NPBF16 = np.dtype(ml_dtypes.bfloat16)

P = 128
NF = 127            # features incl. bias
B = 256             # batch
NCLASS = 16         # width classes (i-rows per core)
WIDTHS = [128 - 8 * t for t in range(NCLASS)]           # 128,120,...,8
OFFS = np.concatenate([[0], np.cumsum(WIDTHS)])          # class col offsets
NCOLS = int(OFFS[-1])                                    # 1088
# chunk = (class range); each chunk is one matmul (N<=512, one PSUM bank)
CHUNKS = [(0, 4), (4, 9), (9, 16)]
CHUNK_COLS = [int(OFFS[hi] - OFFS[lo]) for lo, hi in CHUNKS]      # 464,400,224
# class -> chunk index
CLASS_CHUNK = {t: ci for ci, (lo, hi) in enumerate(CHUNKS) for t in range(lo, hi)}

_CACHE = {}


def _build_nc():
    nc = bacc.Bacc("TRN2", target_bir_lowering=False, debug=False)
    xt = nc.dram_tensor("xt", [P, B], BF16, kind="ExternalInput")      # xf^T padded
    wd = nc.dram_tensor("wd", [P, NCOLS], BF16, kind="ExternalInput")  # dense pair weights
    # xz: cols 0:32 = per-class scalars xf_i (both tiles), 32:160 = tile-0 xf
    # rows; xb1: tile-1 xf rows.  One DMA carries everything the first DVE op
    # needs, so its completion semaphore is the only x-side gate.
    xz = nc.dram_tensor("xz", [P, 32 + P], F32, kind="ExternalInput")
    xb1 = nc.dram_tensor("xb1", [P, P], F32, kind="ExternalInput")
    out = nc.dram_tensor("out", [P, 2], F32, kind="ExternalOutput")

    from contextlib import ExitStack
    with ExitStack() as ctx:
        xt_t = ctx.enter_context(nc.sbuf_tensor("xt_t", [P, B], BF16))
        wd_t = ctx.enter_context(nc.sbuf_tensor("wd_t", [P, NCOLS], BF16))
        xz_t = ctx.enter_context(nc.sbuf_tensor("xz_t", [P, 32 + P], F32))
        xb1_t = ctx.enter_context(nc.sbuf_tensor("xb1_t", [P, P], F32))
        z0_sb = ctx.enter_context(nc.sbuf_tensor("z0_sb", [P, NCOLS], F32))
        z1_sb = ctx.enter_context(nc.sbuf_tensor("z1_sb", [P, NCOLS], F32))
        s_t = ctx.enter_context(nc.sbuf_tensor("s_t", [P, P], F32))
        g_t = ctx.enter_context(nc.sbuf_tensor("g_t", [P, 2 * NCLASS], F32))
        res_t = ctx.enter_context(nc.sbuf_tensor("res_t", [P, 2], F32))
        z00a = ctx.enter_context(nc.psum_tensor("z00a", [P, 248], F32))
        z00b = ctx.enter_context(nc.psum_tensor("z00b", [P, 216], F32))
        z01 = ctx.enter_context(nc.psum_tensor("z01", [P, CHUNK_COLS[1]], F32))
        z02 = ctx.enter_context(nc.psum_tensor("z02", [P, CHUNK_COLS[2]], F32))
        z10 = ctx.enter_context(nc.psum_tensor("z10", [P, CHUNK_COLS[0]], F32))
        z11 = ctx.enter_context(nc.psum_tensor("z11", [P, CHUNK_COLS[1]], F32))
        z12 = ctx.enter_context(nc.psum_tensor("z12", [P, CHUNK_COLS[2]], F32))
        s_xt = ctx.enter_context(nc.semaphore("s_xt"))
        s_wd = ctx.enter_context(nc.semaphore("s_wd"))
        s_xbs = ctx.enter_context(nc.semaphore("s_xbs"))
        s_mm = ctx.enter_context(nc.semaphore("s_mm"))
        s_act = ctx.enter_context(nc.semaphore("s_act"))
        s_dve = ctx.enter_context(nc.semaphore("s_dve"))
        s_out = ctx.enter_context(nc.semaphore("s_out"))

        z_sb = [z0_sb, z1_sb]

        # No nc.Block(): engines end independently (no final all-engine
        # barrier), so the walrus per-engine semaphore-file reset epilogue
        # (~50 sem writes per engine) overlaps the DVE phase on the engines
        # that finish early instead of trailing the whole kernel.  It also
        # re-zeroes our 7 sems for the next execution.
        c0 = CHUNK_COLS[0]

        # DMA issues.  sync ring: weights in 3 pieces so the first (248-col)
        # matmul can start ~0.8us earlier; scalar ring: xt, then xz (scalars +
        # tile-0 xf rows: the sole x-side gate of the first DVE op), then xb1.
        nc.sync.dma_start(wd_t[:, 0:248], wd[:, 0:248]).then_inc(s_wd, 16)
        nc.scalar.dma_start(xt_t[:], xt[:]).then_inc(s_xt, 16)
        nc.sync.dma_start(wd_t[:, 248:c0], wd[:, 248:c0]).then_inc(s_wd, 16)
        nc.scalar.dma_start(xz_t[:], xz[:]).then_inc(s_xbs, 16)
        nc.sync.dma_start(wd_t[:, c0:NCOLS], wd[:, c0:NCOLS]).then_inc(s_wd, 16)
        nc.scalar.dma_start(xb1_t[:], xb1[:]).then_inc(s_xbs, 16)

        # PE: tile-0 in 4 matmuls (first covers classes 0-1 only), tile-1 in 3
        mm_specs = [
            # (psum, wd_lo, wd_hi, bt, wait_wd)
            (z00a, 0, 248, 0, 16),
            (z00b, 248, 464, 0, 32),
            (z01, 464, 864, 0, 48),
            (z02, 864, 1088, 0, None),
            (z10, 0, 464, 1, None),
            (z11, 464, 864, 1, None),
            (z12, 864, 1088, 1, None),
        ]
        nc.tensor.wait_ge(s_xt, 16)
        for zp, lo, hi, bt, wait_wd in mm_specs:
            if wait_wd is not None:
                nc.tensor.wait_ge(s_wd, wait_wd)
            nc.tensor.matmul(
                zp[:],
                xt_t[:, bt * P:(bt + 1) * P],
                wd_t[:, lo:hi],
                start=True, stop=True,
            ).then_inc(s_mm, 1)

        # ACT: PSUM->SBUF copies for tile 1 only (all of tile 0 is consumed
        # straight from PSUM by the DVE to cut pipeline-fill latency).
        for k, (zp, lo, hi, bt) in enumerate(
            [(z10, 0, 464, 1), (z11, 464, 864, 1), (z12, 864, 1088, 1)]
        ):
            nc.scalar.wait_ge(s_mm, 5 + k)
            nc.scalar.copy(z_sb[1][:, lo:hi], zp[:]).then_inc(s_act, 1)

        # DVE: 32 fused per-class ops + one reduce.
        # tile-0 class -> (psum tensor, local col offset, s_mm threshold)
        t0_src = {}
        for t in range(NCLASS):
            o = int(OFFS[t])
            if t < 2:
                t0_src[t] = (z00a, o, 1)
            elif t < 4:
                t0_src[t] = (z00b, o - 248, 2)
            elif t < 9:
                t0_src[t] = (z01, o - 464, 3)
            else:
                t0_src[t] = (z02, o - 864, 4)
        nc.vector.wait_ge(s_xbs, 16)
        for bt in range(2):
            for t in range(NCLASS):
                w = WIDTHS[t]
                o = int(OFFS[t])
                if bt == 0:
                    zp, lo, mm_thr = t0_src[t]
                    if t in (0, 2, 4, 9):
                        nc.vector.wait_ge(s_mm, mm_thr)
                    in0 = zp[:, lo:lo + w]
                    in1 = xz_t[:, 32 + 8 * t:32 + 8 * t + w]
                else:
                    if t == 0:
                        nc.vector.wait_ge(s_xbs, 32)
                    if t in (0, 4, 9):
                        nc.vector.wait_ge(s_act, CLASS_CHUNK[t] + 1)
                    in0 = z_sb[1][:, o:o + w]
                    in1 = xb1_t[:, 8 * t:8 * t + w]
                stt = nc.vector.scalar_tensor_tensor(
                    out=s_t[:, :w],
                    in0=in0,
                    scalar=xz_t[:, bt * NCLASS + t:bt * NCLASS + t + 1],
                    in1=in1,
                    op0=mybir.AluOpType.mult,
                    op1=mybir.AluOpType.mult,
                    accum_out=g_t[:, bt * NCLASS + t:bt * NCLASS + t + 1],
                )
                if bt == 1 and t == NCLASS - 2:
                    # Fire the output-DMA gate two DVE ops before the reduce:
                    # sync's descriptor generation (~0.65us) + DGE delay
                    # (~0.65us) elapse before the DMA engine reads res_t,
                    # by which time the last STT + reduce (~0.75us) are done.
                    stt.then_inc(s_dve, 1)
        nc.vector.reduce_sum(
            res_t[:],
            g_t[:].rearrange("p (b t) -> p b t", b=2),
            axis=mybir.AxisListType.X,
        )

        # output DMA; completion is guaranteed by the NEFF epilogue's
        # per-engine DMA drain, so no explicit s_out wait.
        nc.sync.wait_ge(s_dve, 1)
        nc.sync.dma_start(out[:], res_t[:]).then_inc(s_out, 16)

    nc.compile()
    return nc


def _prep_inputs(x, weights, comb_idx):
    """Host-side layout prep (no FLOPs on the runtime data beyond zero-fill
    scatter): build xf paddings and the per-core dense weight chunks."""
    x = np.ascontiguousarray(np.asarray(x, dtype=np.float32))
    w = np.asarray(weights, dtype=np.float32).ravel()
    ci = np.asarray(comb_idx)
    i_, j_ = ci[:, 0].astype(np.int64), ci[:, 1].astype(np.int64)
    k_ = ci[:, 2].astype(np.int64)

    xf = np.concatenate([np.ones((B, 1), np.float32), x], axis=1)   # [256,127]
    xb0m = np.zeros((P, P), np.float32)      # row p: xf[p, :]
    xb0m[:, :NF] = xf[:P, :]
    xb1m = np.zeros((P, P), np.float32)      # row p: xf[128+p, :]
    xb1m[:, :NF] = xf[P:, :]
    xt = np.zeros((P, B), np.float32)
    xt[:NF, :] = xf.T

    # lex pair-row index of each combo
    ar = np.arange(NF, dtype=np.int64)
    rsp = ar * NF - (ar * (ar - 1)) // 2
    q = rsp[i_] + (j_ - i_)
    Wd = np.zeros((8128, NF), np.float32)
    Wd[q, k_] = w

    xt_bf = xt.astype(NPBF16)

    in_maps = []
    for c in range(8):
        big = np.zeros((P, NCOLS), np.float32)
        xzm = np.zeros((P, 32 + P), np.float32)
        xzm[:, 32:32 + P] = xb0m
        for t in range(NCLASS):
            i = 8 * t + c
            if i > 126:
                continue
            xzm[:, t] = xf[:P, i]
            xzm[:, NCLASS + t] = xf[P:, i]
            p0 = int(rsp[i])
            # cols j in [i,127) hold Wd rows p0..p0+(127-i); leading j in
            # [8t, i) and trailing j=127 stay zero
            o = int(OFFS[t])
            big[:NF, o + (i - 8 * t): o + (127 - 8 * t)] = Wd[p0:p0 + (NF - i)].T
        m = {"xt": xt_bf, "xz": xzm, "xb1": xb1m, "wd": big.astype(NPBF16)}
        in_maps.append(m)
    return in_maps


def _get_nc():
    if "nc" not in _CACHE:
        _CACHE["nc"] = _build_nc()
    return _CACHE["nc"]


def run_spmd(x, weights, comb_idx, trace=False):
    nc = _get_nc()
    in_maps = _prep_inputs(x, weights, comb_idx)
    res = run_bass_kernel_spmd(nc, in_maps, list(range(8)), trace=trace)
    acc = np.zeros((B, 1), np.float64)
    for c in range(8):
        r = res.results[c]["out"].astype(np.float64)   # [128, 2]
        acc[:P, 0] += r[:, 0]
        acc[P:, 0] += r[:, 1]
    return acc.astype(np.float32), res


def kernel(x, weights, comb_idx):
    out, _ = run_spmd(x, weights, comb_idx, trace=False)
    return out


# revision 23
# speedup vs baseline: 1.0985x; 1.0985x over previous
"""HONU order-3 kernel for 8 TRN2 NeuronCores — raw bass (no TileContext).

Math: out[b] = sum_{i<=j<=k} w_ijk * xf_i * xf_j * xf_k,  xf = [1, x] (127 feats).

Restructuring: group combos by pair (i,j) (lex order => per-pair weights are a
contiguous slice of `weights`).  Let W[(i,j), k] = w_ijk for k>=j (0 otherwise).
Then  Z[b,(i,j)] = sum_k W[(i,j),k] * xf[b,k]   (a dense matmul), and
      out[b]     = sum_i xf_i * sum_{j>=i} xf_j * Z[b,(i,j)]
mapped onto one fused scalar_tensor_tensor per i-row and batch tile:
      g[:, t] = sum_j ((Z * xf_i) * xf_j).

Sharding: pair-rows i are dealt round-robin to the 8 cores (core c gets rows
i = 8t + c, t = 0..15); every core runs the same (SPMD) program and returns a
[128, 2] partial (batch-tile column-major) that the host sums across cores.

Why raw bass: the TileContext version spent ~10us in framework preamble +
semaphore-teardown epilogue and ~14us moving 837KB as sub-1KB DMA packets.
Here: 5 consolidated DMAs (>=512B per-partition rows), 7 manual semaphores,
no nc.Block() final barrier (lets the fixed walrus per-engine sem-file-reset
epilogue start as early as possible), bf16 matmul operands (halves weight
DMA; full-rate PE), fp32 elementwise (STT has no 2x bf16 mode - measured),
tile-0 chunks 0-1 consumed straight from PSUM to cut pipeline-fill latency.
Measured: 28.5us (Tile baseline) -> ~20.5us.
"""

import numpy as np
import ml_dtypes

import concourse.bass as bass
import concourse.bacc as bacc
import concourse.mybir as mybir
from concourse.bass_utils import run_bass_kernel_spmd

F32 = mybir.dt.float32
BF16 = mybir.dt.bfloat16
NPBF16 = np.dtype(ml_dtypes.bfloat16)

P = 128
NF = 127            # features incl. bias
B = 256             # batch
NCLASS = 16         # width classes (i-rows per core)
WIDTHS = [128 - 8 * t for t in range(NCLASS)]           # 128,120,...,8
OFFS = np.concatenate([[0], np.cumsum(WIDTHS)])          # class col offsets
NCOLS = int(OFFS[-1])                                    # 1088
# chunk = (class range); each chunk is one matmul (N<=512, one PSUM bank)
CHUNKS = [(0, 4), (4, 9), (9, 16)]
CHUNK_COLS = [int(OFFS[hi] - OFFS[lo]) for lo, hi in CHUNKS]      # 464,400,224
# class -> chunk index
CLASS_CHUNK = {t: ci for ci, (lo, hi) in enumerate(CHUNKS) for t in range(lo, hi)}

_CACHE = {}


def _build_nc():
    nc = bacc.Bacc("TRN2", target_bir_lowering=False, debug=False)
    xt = nc.dram_tensor("xt", [P, B], BF16, kind="ExternalInput")      # xf^T padded
    wd = nc.dram_tensor("wd", [P, NCOLS], BF16, kind="ExternalInput")  # dense pair weights
    # xz: cols 0:32 = per-class scalars xf_i (both tiles), 32:160 = tile-0 xf
    # rows; xb1: tile-1 xf rows.  One DMA carries everything the first DVE op
    # needs, so its completion semaphore is the only x-side gate.
    xz = nc.dram_tensor("xz", [P, 32 + P], F32, kind="ExternalInput")
    xb1 = nc.dram_tensor("xb1", [P, P], F32, kind="ExternalInput")
    out = nc.dram_tensor("out", [P, 2], F32, kind="ExternalOutput")

    from contextlib import ExitStack
    with ExitStack() as ctx:
        xt_t = ctx.enter_context(nc.sbuf_tensor("xt_t", [P, B], BF16))
        wd_t = ctx.enter_context(nc.sbuf_tensor("wd_t", [P, NCOLS], BF16))
        xz_t = ctx.enter_context(nc.sbuf_tensor("xz_t", [P, 32 + P], F32))
        xb1_t = ctx.enter_context(nc.sbuf_tensor("xb1_t", [P, P], F32))
        z0_sb = ctx.enter_context(nc.sbuf_tensor("z0_sb", [P, NCOLS], F32))
        z1_sb = ctx.enter_context(nc.sbuf_tensor("z1_sb", [P, NCOLS], F32))
        s_t = ctx.enter_context(nc.sbuf_tensor("s_t", [P, P], F32))
        g_t = ctx.enter_context(nc.sbuf_tensor("g_t", [P, 2 * NCLASS], F32))
        res_t = ctx.enter_context(nc.sbuf_tensor("res_t", [P, 2], F32))
        z00a = ctx.enter_context(nc.psum_tensor("z00a", [P, 248], F32))
        z00b = ctx.enter_context(nc.psum_tensor("z00b", [P, 216], F32))
        z01 = ctx.enter_context(nc.psum_tensor("z01", [P, CHUNK_COLS[1]], F32))
        z02 = ctx.enter_context(nc.psum_tensor("z02", [P, CHUNK_COLS[2]], F32))
        z10 = ctx.enter_context(nc.psum_tensor("z10", [P, CHUNK_COLS[0]], F32))
        z11 = ctx.enter_context(nc.psum_tensor("z11", [P, CHUNK_COLS[1]], F32))
        z12 = ctx.enter_context(nc.psum_tensor("z12", [P, CHUNK_COLS[2]], F32))
        s_xt = ctx.enter_context(nc.semaphore("s_xt"))
        s_wd = ctx.enter_context(nc.semaphore("s_wd"))
        s_xbs = ctx.enter_context(nc.semaphore("s_xbs"))
        s_mm = ctx.enter_context(nc.semaphore("s_mm"))
        s_act = ctx.enter_context(nc.semaphore("s_act"))
        s_dve = ctx.enter_context(nc.semaphore("s_dve"))
        s_out = ctx.enter_context(nc.semaphore("s_out"))

        z_sb = [z0_sb, z1_sb]

        # No nc.Block(): engines end independently (no final all-engine
        # barrier), so the walrus per-engine semaphore-file reset epilogue
        # (~50 sem writes per engine) overlaps the DVE phase on the engines
        # that finish early instead of trailing the whole kernel.  It also
        # re-zeroes our 7 sems for the next execution.
        c0 = CHUNK_COLS[0]

        # DMA issues.  sync ring: weights in 3 pieces so the first (248-col)
        # matmul can start ~0.8us earlier; scalar ring: xt, then xz (scalars +
        # tile-0 xf rows: the sole x-side gate of the first DVE op), then xb1.
        nc.sync.dma_start(wd_t[:, 0:248], wd[:, 0:248]).then_inc(s_wd, 16)
        nc.scalar.dma_start(xt_t[:], xt[:]).then_inc(s_xt, 16)
        nc.sync.dma_start(wd_t[:, 248:c0], wd[:, 248:c0]).then_inc(s_wd, 16)
        nc.scalar.dma_start(xz_t[:], xz[:]).then_inc(s_xbs, 16)
        nc.sync.dma_start(wd_t[:, c0:NCOLS], wd[:, c0:NCOLS]).then_inc(s_wd, 16)
        nc.scalar.dma_start(xb1_t[:], xb1[:]).then_inc(s_xbs, 16)

        # PE: tile-0 in 4 matmuls (first covers classes 0-1 only), tile-1 in 3
        mm_specs = [
            # (psum, wd_lo, wd_hi, bt, wait_wd)
            (z00a, 0, 248, 0, 16),
            (z00b, 248, 464, 0, 32),
            (z01, 464, 864, 0, 48),
            (z02, 864, 1088, 0, None),
            (z10, 0, 464, 1, None),
            (z11, 464, 864, 1, None),
            (z12, 864, 1088, 1, None),
        ]
        nc.tensor.wait_ge(s_xt, 16)
        for zp, lo, hi, bt, wait_wd in mm_specs:
            if wait_wd is not None:
                nc.tensor.wait_ge(s_wd, wait_wd)
            nc.tensor.matmul(
                zp[:],
                xt_t[:, bt * P:(bt + 1) * P],
                wd_t[:, lo:hi],
                start=True, stop=True,
            ).then_inc(s_mm, 1)

        # ACT: PSUM->SBUF copies for tile 1 only (all of tile 0 is consumed
        # straight from PSUM by the DVE to cut pipeline-fill latency).
        for k, (zp, lo, hi, bt) in enumerate(
            [(z10, 0, 464, 1), (z11, 464, 864, 1), (z12, 864, 1088, 1)]
        ):
            nc.scalar.wait_ge(s_mm, 5 + k)
            nc.scalar.copy(z_sb[1][:, lo:hi], zp[:]).then_inc(s_act, 1)

        # DVE: 32 fused per-class ops + one reduce.
        # tile-0 class -> (psum tensor, local col offset, s_mm threshold)
        t0_src = {}
        for t in range(NCLASS):
            o = int(OFFS[t])
            if t < 2:
                t0_src[t] = (z00a, o, 1)
            elif t < 4:
                t0_src[t] = (z00b, o - 248, 2)
            elif t < 9:
                t0_src[t] = (z01, o - 464, 3)
            else:
                t0_src[t] = (z02, o - 864, 4)
        nc.vector.wait_ge(s_xbs, 16)
        for bt in range(2):
            for t in range(NCLASS):
                w = WIDTHS[t]
                o = int(OFFS[t])
                if bt == 0:
                    zp, lo, mm_thr = t0_src[t]
                    if t in (0, 2, 4, 9):
                        nc.vector.wait_ge(s_mm, mm_thr)
                    in0 = zp[:, lo:lo + w]
                    in1 = xz_t[:, 32 + 8 * t:32 + 8 * t + w]
                else:
                    if t == 0:
                        nc.vector.wait_ge(s_xbs, 32)
                    if t in (0, 4, 9):
                        nc.vector.wait_ge(s_act, CLASS_CHUNK[t] + 1)
                    in0 = z_sb[1][:, o:o + w]
                    in1 = xb1_t[:, 8 * t:8 * t + w]
                nc.vector.scalar_tensor_tensor(
                    out=s_t[:, :w],
                    in0=in0,
                    scalar=xz_t[:, bt * NCLASS + t:bt * NCLASS + t + 1],
                    in1=in1,
                    op0=mybir.AluOpType.mult,
                    op1=mybir.AluOpType.mult,
                    accum_out=g_t[:, bt * NCLASS + t:bt * NCLASS + t + 1],
                )
        nc.vector.reduce_sum(
            res_t[:],
            g_t[:].rearrange("p (b t) -> p b t", b=2),
            axis=mybir.AxisListType.X,
        ).then_inc(s_dve, 1)

        # output DMA; completion is guaranteed by the NEFF epilogue's
        # per-engine DMA drain, so no explicit s_out wait.
        nc.sync.wait_ge(s_dve, 1)
        nc.sync.dma_start(out[:], res_t[:]).then_inc(s_out, 16)

    nc.compile()
    return nc


def _prep_inputs(x, weights, comb_idx):
    """Host-side layout prep (no FLOPs on the runtime data beyond zero-fill
    scatter): build xf paddings and the per-core dense weight chunks."""
    x = np.ascontiguousarray(np.asarray(x, dtype=np.float32))
    w = np.asarray(weights, dtype=np.float32).ravel()
    ci = np.asarray(comb_idx)
    i_, j_ = ci[:, 0].astype(np.int64), ci[:, 1].astype(np.int64)
    k_ = ci[:, 2].astype(np.int64)

    xf = np.concatenate([np.ones((B, 1), np.float32), x], axis=1)   # [256,127]
    xb0m = np.zeros((P, P), np.float32)      # row p: xf[p, :]
    xb0m[:, :NF] = xf[:P, :]
    xb1m = np.zeros((P, P), np.float32)      # row p: xf[128+p, :]
    xb1m[:, :NF] = xf[P:, :]
    xt = np.zeros((P, B), np.float32)
    xt[:NF, :] = xf.T

    # lex pair-row index of each combo
    ar = np.arange(NF, dtype=np.int64)
    rsp = ar * NF - (ar * (ar - 1)) // 2
    q = rsp[i_] + (j_ - i_)
    Wd = np.zeros((8128, NF), np.float32)
    Wd[q, k_] = w

    xt_bf = xt.astype(NPBF16)

    in_maps = []
    for c in range(8):
        big = np.zeros((P, NCOLS), np.float32)
        xzm = np.zeros((P, 32 + P), np.float32)
        xzm[:, 32:32 + P] = xb0m
        for t in range(NCLASS):
            i = 8 * t + c
            if i > 126:
                continue
            xzm[:, t] = xf[:P, i]
            xzm[:, NCLASS + t] = xf[P:, i]
            p0 = int(rsp[i])
            # cols j in [i,127) hold Wd rows p0..p0+(127-i); leading j in
            # [8t, i) and trailing j=127 stay zero
            o = int(OFFS[t])
            big[:NF, o + (i - 8 * t): o + (127 - 8 * t)] = Wd[p0:p0 + (NF - i)].T
        m = {"xt": xt_bf, "xz": xzm, "xb1": xb1m, "wd": big.astype(NPBF16)}
        in_maps.append(m)
    return in_maps


def _get_nc():
    if "nc" not in _CACHE:
        _CACHE["nc"] = _build_nc()
    return _CACHE["nc"]


def run_spmd(x, weights, comb_idx, trace=False):
    nc = _get_nc()
    in_maps = _prep_inputs(x, weights, comb_idx)
    res = run_bass_kernel_spmd(nc, in_maps, list(range(8)), trace=trace)
    acc = np.zeros((B, 1), np.float64)
    for c in range(8):
        r = res.results[c]["out"].astype(np.float64)   # [128, 2]
        acc[:P, 0] += r[:, 0]
        acc[P:, 0] += r[:, 1]
    return acc.astype(np.float32), res


def kernel(x, weights, comb_idx):
    out, _ = run_spmd(x, weights, comb_idx, trace=False)
    return out
